# revision 6
# baseline (speedup 1.0000x reference)
"""Trainium2 Bass kernel for nn_ConditionalNeuralSDEGenerator.

Strategy: data-parallel over batch (256 -> 32 per core, 8 cores).
Host (numpy): initial-condition MLPs + 127-step rsig scan (tiny, O(ms)),
Brownian-increment prep, weight fold/transpose, dW partition-broadcast.
Device: the 511-step sequential SDE reservoir scan (the dominant cost)
with the readout fused in every 16 steps.

Device layout: state kept transposed S[r, b] as one [128, 64] tile per
step (m-half h of rows: cols h*32..h*32+32 hold rows h*128..h*128+128),
living in a 32-slot SBUF ring. Per step: 8 matmuls [128,128]x[128,32]
(weights stationary), 4 tanh activations with per-partition bias, 3 DVE
tensor ops. Readout x[t,b] = Wr @ S_t runs as 2 thin matmuls per 16-step
block against the ring, accumulated in PSUM and DMA'd to DRAM.
"""

import sys

if "/opt/trn_rl_repo" not in sys.path:
    sys.path.insert(0, "/opt/trn_rl_repo")

import numpy as np

import concourse.bacc as bacc
import concourse.bass as bass
import concourse.mybir as mybir
import concourse.tile as tile
from concourse.bass_utils import run_bass_kernel_spmd

# ---- problem constants (hardcoded per contest rules) ----
BATCH = 256
N_LAGS = 512
RD = 256
N_CORES = 8
B_LOC = BATCH // N_CORES          # 32
NSTEP = N_LAGS - 1                # 511
NSTATE = N_LAGS                   # 512 states incl. R0
RING = 32                         # ring slots (>= 2*BLK)
BLK = 16                          # readout block (steps per readout matmul pair)
NBLK = NSTATE // BLK              # 32
SC = 2 * B_LOC                    # 64 state cols per step slot
DW_PAD = 512                      # dW buffer padded to 512 steps

F32 = mybir.dt.float32
USE_FP16 = False                  # GEMM/readout dtype: False -> fp32 everywhere
DT = mybir.dt.float16 if USE_FP16 else F32
NP_DT = np.float16 if USE_FP16 else np.float32

_CACHE = {}


def _build_bass():
    nc = bacc.Bacc(None, target_bir_lowering=False)

    # ---- DRAM I/O ----
    w1 = nc.dram_tensor("w1", [128, 4, 128], DT, kind="ExternalInput")
    w2 = nc.dram_tensor("w2", [128, 4, 128], DT, kind="ExternalInput")
    c1 = nc.dram_tensor("c1", [128, 2], F32, kind="ExternalInput")
    c2 = nc.dram_tensor("c2", [128, 2], F32, kind="ExternalInput")
    wr = nc.dram_tensor("wr", [128, 2], DT, kind="ExternalInput")
    r0 = nc.dram_tensor("r0", [128, SC], F32, kind="ExternalInput")
    dwbc = nc.dram_tensor("dwbc", [128, DW_PAD * SC], DT, kind="ExternalInput")
    xout = nc.dram_tensor("xout", [NBLK, BLK * B_LOC], F32, kind="ExternalOutput")

    N_DWT = 8                      # dW chunk tiles
    DW_CHUNK = DW_PAD // N_DWT     # 64 steps per chunk

    with tile.TileContext(nc) as tc:
        with (
            tc.tile_pool(name="const", bufs=1) as cpool,
            tc.tile_pool(name="dwp", bufs=1) as dwpool,
            tc.tile_pool(name="ring", bufs=1) as rpool,
            tc.tile_pool(name="work", bufs=3) as wpool,
            tc.tile_pool(name="psum", bufs=2, space="PSUM") as ppool,
            tc.tile_pool(name="rops", bufs=2, space="PSUM") as ropool,
        ):
            w1sb = cpool.tile([128, 4, 128], DT)
            w2sb = cpool.tile([128, 4, 128], DT)
            c1sb = cpool.tile([128, 2], F32)
            c2sb = cpool.tile([128, 2], F32)
            wrsb = cpool.tile([128, 2], DT)
            nc.sync.dma_start(w1sb[:], w1[:])
            nc.sync.dma_start(w2sb[:], w2[:])
            nc.sync.dma_start(c1sb[:], c1[:])
            nc.sync.dma_start(c2sb[:], c2[:])
            nc.sync.dma_start(wrsb[:], wr[:])

            ring = rpool.tile([128, RING * SC], F32)
            nc.sync.dma_start(ring[:, 0:SC], r0[:])
            if USE_FP16:
                ring16 = rpool.tile([128, RING * SC], DT)
                nc.vector.tensor_copy(ring16[:, 0:SC], ring[:, 0:SC])
                gring = ring16
            else:
                gring = ring

            dwt = []
            for i in range(N_DWT):
                t = dwpool.tile([128, DW_CHUNK * SC], DT, name=f"dwt{i}")
                nc.sync.dma_start(t[:], dwbc[:, i * DW_CHUNK * SC:(i + 1) * DW_CHUNK * SC])
                dwt.append(t)

            # warm up the tanh table load early (output unused)
            warm = wpool.tile([128, 2], F32, tag="warm", bufs=1)
            nc.scalar.activation(warm[:], c1sb[:], mybir.ActivationFunctionType.Tanh)

            ring_v = gring.rearrange("p (s c) -> p s c", c=SC)

            for t in range(NSTEP):
                s_in = t % RING
                s_out = (t + 1) % RING
                pd = ppool.tile([128, SC], F32, tag="pdrift")
                pf = ppool.tile([128, SC], F32, tag="pdiffu")
                rhs0 = gring[:, s_in * SC: s_in * SC + B_LOC]
                rhs1 = gring[:, s_in * SC + B_LOC: s_in * SC + SC]
                for m in (0, 1):
                    o = pd[:, m * B_LOC:(m + 1) * B_LOC]
                    nc.tensor.matmul(o, w1sb[:, 2 * m + 0, :], rhs0, start=True, stop=False)
                    nc.tensor.matmul(o, w1sb[:, 2 * m + 1, :], rhs1, start=False, stop=True)
                for m in (0, 1):
                    o = pf[:, m * B_LOC:(m + 1) * B_LOC]
                    nc.tensor.matmul(o, w2sb[:, 2 * m + 0, :], rhs0, start=True, stop=False)
                    nc.tensor.matmul(o, w2sb[:, 2 * m + 1, :], rhs1, start=False, stop=True)

                drift = wpool.tile([128, SC], F32, tag="drift")
                diffu = wpool.tile([128, SC], F32, tag="diffu")
                for m in (0, 1):
                    nc.scalar.activation(
                        drift[:, m * B_LOC:(m + 1) * B_LOC],
                        pd[:, m * B_LOC:(m + 1) * B_LOC],
                        mybir.ActivationFunctionType.Tanh,
                        bias=c1sb[:, m:m + 1],
                    )
                for m in (0, 1):
                    nc.scalar.activation(
                        diffu[:, m * B_LOC:(m + 1) * B_LOC],
                        pf[:, m * B_LOC:(m + 1) * B_LOC],
                        mybir.ActivationFunctionType.Tanh,
                        bias=c2sb[:, m:m + 1],
                    )

                v = wpool.tile([128, SC], F32, tag="v")
                u = wpool.tile([128, SC], F32, tag="u")
                nc.vector.tensor_add(v, ring[:, s_in * SC:(s_in + 1) * SC], drift)
                dws = dwt[t // DW_CHUNK][:, (t % DW_CHUNK) * SC:(t % DW_CHUNK + 1) * SC]
                nc.vector.tensor_mul(u, diffu, dws)
                nc.vector.tensor_add(ring[:, s_out * SC:(s_out + 1) * SC], v, u)
                if USE_FP16:
                    nc.vector.tensor_copy(
                        gring[:, s_out * SC:(s_out + 1) * SC],
                        ring[:, s_out * SC:(s_out + 1) * SC],
                    )

                s = t + 1
                if s % BLK == BLK - 1:
                    blk = s // BLK
                    base = (blk * BLK) % RING
                    ro = ropool.tile([1, BLK * B_LOC], F32, tag="ro")
                    for m in (0, 1):
                        nc.tensor.matmul(
                            ro[0:1, :],
                            wrsb[:, m:m + 1],
                            ring_v[:, base:base + BLK, m * B_LOC:(m + 1) * B_LOC],
                            start=(m == 0),
                            stop=(m == 1),
                        )
                    ro_sb = wpool.tile([1, BLK * B_LOC], F32, tag="rosb", bufs=2)
                    nc.vector.tensor_copy(ro_sb[0:1, :], ro[0:1, :])
                    nc.sync.dma_start(xout[blk:blk + 1, :], ro_sb[0:1, :])

    nc.compile()
    return nc


def _host_prep(inputs):
    """All the small sequential/setup math, in fp32 numpy (matches reference)."""
    f32 = np.float32
    x_past = np.asarray(inputs["x_past"], f32)
    V_noise = np.asarray(inputs["V_noise"], f32)
    inc = np.asarray(inputs["increments"], f32)
    act = np.tanh

    V = act(V_noise @ np.asarray(inputs["W1h"], f32).T + np.asarray(inputs["b1h"], f32))
    V = V @ np.asarray(inputs["W1o"], f32).T + np.asarray(inputs["b1o"], f32)

    dx = x_past[1:, 0] - x_past[:-1, 0]
    A1 = np.asarray(inputs["A1"], f32)
    A2 = np.asarray(inputs["A2"], f32)
    xi1 = np.asarray(inputs["xi1"], f32)
    xi2 = np.asarray(inputs["xi2"], f32)
    Z = np.zeros(RD, f32)
    for i in range(dx.shape[0]):
        Z = (Z + act(A1 @ Z + xi1) + act(A2 @ Z + xi2) * dx[i]).astype(f32)
    rsig = act(Z @ np.asarray(inputs["W2h"], f32).T + np.asarray(inputs["b2h"], f32))
    rsig = rsig @ np.asarray(inputs["W2o"], f32).T + np.asarray(inputs["b2o"], f32)

    B = V_noise.shape[0]
    R0 = np.concatenate(
        [np.broadcast_to(rsig[None, :], (B, rsig.shape[0])), V], axis=1
    ).astype(f32)

    Wp = np.cumsum(inc, axis=1, dtype=f32)
    Wp[:, 0, :] = 0.0
    dW = (Wp[:, 1:, 0] - Wp[:, :-1, 0]).astype(f32)  # (B, NSTEP)

    rho1, rho2, rho3, rho4 = (f32(inputs[k]) for k in ("rho1", "rho2", "rho3", "rho4"))
    B1s = (rho1 * np.asarray(inputs["B1"], f32)).astype(f32)
    B2s = (rho3 * np.asarray(inputs["B2"], f32)).astype(f32)
    c1 = (rho2 * np.asarray(inputs["lam1"], f32)).astype(f32)
    c2 = (rho4 * np.asarray(inputs["lam2"], f32)).astype(f32)
    Wr = np.asarray(inputs["Wr"], f32)[0]
    br = f32(np.asarray(inputs["br"], f32)[0])
    return R0, dW, B1s, B2s, c1, c2, Wr, br


def _weight_blocks(Bs):
    """(rho*B) -> [128, 4, 128] lhsT blocks: blk[p, 2m+k, j] = Bs[m*128+j, k*128+p]."""
    BT = np.ascontiguousarray(Bs.T)  # [r_in, r_out]
    out = np.empty((128, 4, 128), np.float32)
    for k in (0, 1):
        for m in (0, 1):
            out[:, 2 * m + k, :] = BT[k * 128:(k + 1) * 128, m * 128:(m + 1) * 128]
    return out


def kernel(**inputs) -> np.ndarray:
    R0, dW, B1s, B2s, c1, c2, Wr, br = _host_prep(inputs)

    w1_blocks = _weight_blocks(B1s).astype(NP_DT)
    w2_blocks = _weight_blocks(B2s).astype(NP_DT)
    c1_arr = np.ascontiguousarray(c1.reshape(2, 128).T)          # [128, 2]
    c2_arr = np.ascontiguousarray(c2.reshape(2, 128).T)
    wr_arr = np.ascontiguousarray(Wr.reshape(2, 128).T).astype(NP_DT)

    in_maps = []
    for core in range(N_CORES):
        bs = slice(core * B_LOC, (core + 1) * B_LOC)
        # r0 tile: [128, 64]; col m*32+b = R0[b, m*128+p]
        r0c = R0[bs]                                              # [32, 256]
        r0_tile = np.empty((128, SC), np.float32)
        for m in (0, 1):
            r0_tile[:, m * B_LOC:(m + 1) * B_LOC] = r0c[:, m * 128:(m + 1) * 128].T
        # dwbc: [128, 512*64]; col t*64 + c = dW[c%32, t] broadcast over partitions
        dwc = dW[bs]                                              # [32, 511]
        dwp = np.zeros((DW_PAD, SC), np.float32)
        dwp[:NSTEP, :B_LOC] = dwc.T
        dwp[:NSTEP, B_LOC:] = dwc.T
        dw_tile = np.ascontiguousarray(
            np.broadcast_to(dwp.reshape(1, -1), (128, DW_PAD * SC))
        ).astype(NP_DT)
        in_maps.append({
            "w1": w1_blocks, "w2": w2_blocks,
            "c1": c1_arr, "c2": c2_arr, "wr": wr_arr,
            "r0": r0_tile, "dwbc": dw_tile,
        })

    if "nc" not in _CACHE:
        _CACHE["nc"] = _build_bass()
    nc = _CACHE["nc"]

    res = run_bass_kernel_spmd(nc, in_maps, core_ids=list(range(N_CORES)))

    out = np.empty((BATCH, N_LAGS, 1), np.float32)
    for core in range(N_CORES):
        xo = res.results[core]["xout"]                            # [32, 512]
        xtb = xo.reshape(NBLK * BLK, B_LOC)                       # [512, 32] (t, b)
        out[core * B_LOC:(core + 1) * B_LOC, :, 0] = xtb.T + br
    return out
